# revision 1
# baseline (speedup 1.0000x reference)
"""Trainium2 Bass kernel for nn_Attention_11287174054323.

Full attention layer: QKV projections + RoPE + softmax attention + output
projection.  B=2, S=2048, DIM=2048, 16 heads x 128 head_dim, fp32 I/O.

Sharding: tensor-parallel over heads across 8 NeuronCores (2 heads/core).
Each core computes q/k/v projections for its head slice, full attention for
its heads, and a partial output projection (row slice of Wo); the host sums
the 8 partials.

Per-core layout strategy:
  - x is passed pre-transposed (xT [DIM, B*S]) so projections can contract
    over DIM on the partition axis.
  - Q^T/K^T are produced in [head_dim, token] layout; RoPE is fused into the
    PSUM eviction (rotate-half via cross-partition-write multiplies).
  - Scores are computed transposed (S^T = K @ Q^T), softmax-normalization is
    deferred past the A^T = V^T @ exp(S^T) accumulation (linearity); the
    denominator is accumulated with DVE adds, reduced+broadcast with
    ones-matmuls in PSUM, inverted with reciprocal_approx_fast.
  - Projections/scores run in float32r (tf32-like; measured faster than
    fp16/bf16 in this kernel's steady state).  The exp/AV group (e, acc,
    v, ones) and the out-projection operands (at, wo) are fp16: matmul
    rate is unchanged but the DVE denominator adds drop 1220ns -> 689ns
    (halved SBUF traffic under contention) and DMA shrinks.
  - Weights/x are passed pre-rearranged to partition-contiguous layouts
    (strided gather DMAs cost 10-18us of issue time each, ~110GB/s).
  - Phase 1 streams x on the ACT engine's DMA queue in parallel with
    weights on SP; out-projection PSUM evictions are split ~1.5 ACT /
    2.5 DVE to balance engine load; the last attention chunk is split in
    two so the final out-projection chunks start earlier.
  - The partial output is staged to fp16 and summed on the host in fp32
    (halves the output DMA stream).

Measured on HW: 461.6us (fp32r baseline) -> 414.4us.
"""

import sys

sys.path.insert(0, "/opt/trn_rl_repo")

import numpy as np

import concourse.tile as tile
import concourse.mybir as mybir
from concourse import bacc
from concourse.bass_utils import run_bass_kernel_spmd

P = 128
B, S, DIM = 2, 2048, 2048
TOK = B * S                     # 4096 tokens
HEADS_PER_CORE = 2
INNER_C = HEADS_PER_CORE * P    # 256 per-core inner dim
KC = DIM // P                   # 16 contraction chunks
TC = 512                        # phase-1 token chunk
NTC = TOK // TC                 # 8
IC = 512                        # attention i-chunk (queries)
NIC = S // IC                   # 4 per (batch, head)
NJC = S // P                    # 16 key chunks per (batch, head)
SCALE = float(P) ** -0.5

F32 = mybir.dt.float32
F16 = mybir.dt.float16
MM = mybir.dt.float32r          # matmul dtype for projections/scores
AVD = mybir.dt.float16          # dtype of the exp/AV group (e, acc, v, ones)
AVD_NP = np.float16

N_CORES = 8
Mul = mybir.AluOpType.mult


def _build():
    nc = bacc.Bacc("TRN2", target_bir_lowering=False)

    # weights/activations arrive pre-rearranged from the host so every DMA
    # is contiguous per partition (strided gathers cost 10-16us to issue)
    xT_d = nc.dram_tensor("xT", [NTC, P, KC, TC], MM, kind="ExternalInput")
    wq_d = nc.dram_tensor("wq", [P, KC, INNER_C], MM, kind="ExternalInput")
    wk_d = nc.dram_tensor("wk", [P, KC, INNER_C], MM, kind="ExternalInput")
    wv_d = nc.dram_tensor("wv", [P, KC, INNER_C], MM, kind="ExternalInput")
    wo_d = nc.dram_tensor("wo", [P, 2, DIM], F16, kind="ExternalInput")
    cos_d = nc.dram_tensor("cosT", [P, TOK], F32, kind="ExternalInput")
    sin_d = nc.dram_tensor("sinZ", [P, TOK], F32, kind="ExternalInput")
    ones_d = nc.dram_tensor("ones", [P, P], AVD, kind="ExternalInput")
    o_d = nc.dram_tensor("o_part", [TOK, DIM], F16, kind="ExternalOutput")


    with tile.TileContext(nc) as tc:
        with tc.tile_pool(name="persist", bufs=1) as persist, \
             tc.tile_pool(name="dram", bufs=1, space="DRAM") as dram:
            # fp16: attention-window fp16 matmuls measure 216ns vs 230ns
            # fp32r (the in-situ fp16 penalty is phase-1-specific)
            qt = [persist.tile([P, TOK], F16, tag=f"qt{h}", name=f"qt{h}")
                  for h in range(2)]
            kt = [persist.tile([P, TOK], F16, tag=f"kt{h}", name=f"kt{h}")
                  for h in range(2)]
            ones_t = persist.tile([P, P], AVD, tag="ones")
            v_dram = dram.tile([TOK, INNER_C], AVD)

            # ---------------- Phase 1: QKV projections + RoPE ----------
            with tc.tile_pool(name="w1", bufs=1) as wpool, \
                 tc.tile_pool(name="x1", bufs=2) as xpool, \
                 tc.tile_pool(name="ev1", bufs=2) as evpool, \
                 tc.tile_pool(name="ps1", bufs=1, space="PSUM") as psum1:
                wq_t = wpool.tile([P, KC, INNER_C], MM, tag="wq")
                wk_t = wpool.tile([P, KC, INNER_C], MM, tag="wk")
                wv_t = wpool.tile([P, KC, INNER_C], MM, tag="wv")
                # Two parallel DMA queues in phase 1: SP streams weights,
                # the (otherwise idle) ACT engine queue streams x chunks.
                nc.sync.dma_start(wq_t[:, 0:1, :], wq_d[:, 0:1, :])
                xt0 = xpool.tile([P, KC, TC], MM, tag="xt")
                nc.scalar.dma_start(xt0[:, 0:2, :], xT_d[0, :, 0:2, :])
                nc.sync.dma_start(wq_t[:, 1:4, :], wq_d[:, 1:4, :])
                nc.scalar.dma_start(xt0[:, 2:KC // 2, :],
                                    xT_d[0, :, 2:KC // 2, :])
                nc.sync.dma_start(wq_t[:, 4:KC, :], wq_d[:, 4:KC, :])
                nc.scalar.dma_start(xt0[:, KC // 2:KC, :],
                                    xT_d[0, :, KC // 2:KC, :])
                cos0 = evpool.tile([P, TC], F32, tag="cos")
                sin0 = evpool.tile([P, TC], F32, tag="sin")
                nc.sync.dma_start(wk_t[:], wk_d[:])
                nc.scalar.dma_start(cos0[:], cos_d[:, 0:TC])
                nc.scalar.dma_start(sin0[:], sin_d[:, 0:TC])
                nc.sync.dma_start(wv_t[:], wv_d[:])
                nc.sync.dma_start(ones_t[:], ones_d[:])

                for tcn in range(NTC):
                    tsl = slice(tcn * TC, (tcn + 1) * TC)
                    if tcn == 0:
                        xt, cos_t, sin_t = xt0, cos0, sin0
                    else:
                        xt = xpool.tile([P, KC, TC], MM, tag="xt")
                        nc.scalar.dma_start(xt[:], xT_d[tcn])
                        cos_t = evpool.tile([P, TC], F32, tag="cos")
                        sin_t = evpool.tile([P, TC], F32, tag="sin")
                        nc.sync.dma_start(cos_t[:], cos_d[:, tsl])
                        nc.sync.dma_start(sin_t[:], sin_d[:, tsl])

                    # Q^T / K^T chunks with fused RoPE eviction
                    for wt, dsts, nm in ((wq_t, qt, "q"), (wk_t, kt, "k")):
                        for m in range(2):
                            ps = psum1.tile([P, TC], F32, tag=f"ps_{nm}{m}")
                            for kc in range(KC):
                                nc.tensor.matmul(
                                    ps[:], wt[:, kc, m * P:(m + 1) * P],
                                    xt[:, kc, :],
                                    start=(kc == 0), stop=(kc == KC - 1))
                            # rope: dst = ps*cos + rotate_half(ps)*sin
                            tcos = evpool.tile([P, TC], F32, tag="tcos")
                            nc.vector.tensor_mul(tcos[:], ps[:], cos_t[:])
                            tsin = evpool.tile([P, TC], F32, tag="tsin")
                            nc.vector.scalar_tensor_tensor(
                                tsin[0:64, :], ps[64:128, :], 1.0,
                                sin_t[64:128, :], Mul, Mul)
                            nc.vector.scalar_tensor_tensor(
                                tsin[64:128, :], ps[0:64, :], 1.0,
                                sin_t[0:64, :], Mul, Mul)
                            nc.vector.tensor_add(dsts[m][:, tsl], tcos[:],
                                                 tsin[:])

                    # V chunks (tokens on partitions) -> DRAM scratch
                    for m in range(TC // P):
                        ps = psum1.tile([P, INNER_C], F32, tag=f"ps_v{m}")
                        for kc in range(KC):
                            nc.tensor.matmul(
                                ps[:], xt[:, kc, m * P:(m + 1) * P],
                                wv_t[:, kc, :],
                                start=(kc == 0), stop=(kc == KC - 1))
                        vst = evpool.tile([P, INNER_C], AVD, tag="vst")
                        nc.scalar.copy(vst[:], ps[:])
                        r0 = tcn * TC + m * P
                        nc.sync.dma_start(v_dram[r0:r0 + P, :], vst[:])

            # ---------- Phases 2+3: attention + output projection ------
            with tc.tile_pool(name="at", bufs=1) as atpool, \
                 tc.tile_pool(name="vbh", bufs=2) as vpool, \
                 tc.tile_pool(name="e2", bufs=4) as epool, \
                 tc.tile_pool(name="sm2", bufs=2) as smpool, \
                 tc.tile_pool(name="p3", bufs=1) as p3pool, \
                 tc.tile_pool(name="st3", bufs=3) as stpool, \
                 tc.tile_pool(name="ps2", bufs=1, space="PSUM") as psum2, \
                 tc.tile_pool(name="ps3", bufs=2, space="PSUM") as psum3:
                at = [atpool.tile([P, TOK], F16, tag=f"at{h}", name=f"at{h}")
                      for h in range(2)]

                def load_vbh(b, h):
                    boff = b * S
                    vbh = vpool.tile([P, NJC, P], AVD, tag="vbh")
                    nc.sync.dma_start(
                        vbh[:],
                        v_dram[boff:boff + S, h * P:(h + 1) * P]
                        .rearrange("(c p) d -> p c d", p=P))
                    return vbh

                # vbh(0,0) before the (large) wo load: the first AV matmul
                # needs it ~1us into the phase, while wo is only needed at
                # the first ph3_tn (~60us later).
                vbh00 = load_vbh(0, 0)
                wo_t = p3pool.tile([P, 2, DIM], F16, tag="wo")
                nc.sync.dma_start(wo_t[:], wo_d[:])

                def attn_ic(b, h, i0, iw, vbh):
                    """One iw-query chunk (batch offset i0) of attention for
                    (batch, head)."""
                    boff = b * S
                    isl = slice(boff + i0, boff + i0 + iw)
                    ps_at = psum2.tile([P, IC], F32, tag="ps_at", bufs=2)
                    acc = smpool.tile([P, 2 * IC], AVD, tag="acc")
                    # software-pipelined (2x S) -> exp -> (A, colsum) chain
                    NJP = NJC // 2
                    es = [None] * NJP

                    def sp_step(p):
                        ps_s = psum2.tile([P, 2 * IC], F32, tag="ps_s", bufs=2)
                        for half in range(2):
                            jc = 2 * p + half
                            jsl = slice(boff + jc * P, boff + (jc + 1) * P)
                            nc.tensor.matmul(
                                ps_s[:, half * iw:(half + 1) * iw],
                                kt[h][:, jsl], qt[h][:, isl],
                                start=True, stop=True)
                        e = epool.tile([P, 2 * IC], AVD, tag="e")
                        nc.scalar.activation(
                            e[:, 0:2 * iw], ps_s[:, 0:2 * iw],
                            mybir.ActivationFunctionType.Exp,
                            scale=SCALE)
                        es[p] = e

                    def a_step(p):
                        e = es[p]
                        for half in range(2):
                            jc = 2 * p + half
                            eh = e[:, half * iw:(half + 1) * iw]
                            nc.tensor.matmul(ps_at[:, 0:iw], vbh[:, jc, :], eh,
                                             start=(jc == 0),
                                             stop=(jc == NJC - 1))
                        def dv(t):
                            return t.bitcast(F32) if AVD == mybir.dt.float32r else t

                        if p == 0:
                            nc.vector.tensor_copy(acc[:, 0:2 * iw],
                                                  dv(e[:, 0:2 * iw]))
                        else:
                            nc.vector.tensor_add(acc[:, 0:2 * iw],
                                                 dv(acc[:, 0:2 * iw]),
                                                 dv(e[:, 0:2 * iw]))

                    sp_step(0)
                    for p in range(NJP):
                        if p + 1 < NJP:
                            sp_step(p + 1)
                        a_step(p)

                    ps_bc = psum3.tile([P, IC], F32, tag="ps_o")
                    nc.tensor.matmul(ps_bc[:, 0:iw], ones_t[:], acc[:, 0:iw],
                                     start=True, stop=False)
                    nc.tensor.matmul(ps_bc[:, 0:iw], ones_t[:],
                                     acc[:, iw:2 * iw],
                                     start=False, stop=True)
                    recip = smpool.tile([P, IC], F32, tag="recip")
                    nc.vector.reciprocal_approx_fast(recip[:, 0:iw],
                                                     ps_bc[:, 0:iw])
                    nc.vector.tensor_mul(at[h][:, isl], ps_at[:, 0:iw],
                                         recip[:, 0:iw])

                def ph3_tn(tn):
                    """One 128-token chunk of the output projection."""
                    stage = stpool.tile([P, DIM], F16, tag="stage")
                    for en in range(DIM // IC):
                        ps = psum3.tile([P, IC], F32, tag="ps_o")
                        esl = slice(en * IC, (en + 1) * IC)
                        for h in range(2):
                            nc.tensor.matmul(
                                ps[:], at[h][:, tn * P:(tn + 1) * P],
                                wo_t[:, h, esl],
                                start=(h == 0), stop=(h == 1))
                        # ~1.5 ACT / 2.5 DVE per tn balances per-ic ACT
                        # (exp-heavy) vs DVE (adds-heavy) load in b1; the
                        # post-attention tail tns use both engines evenly
                        if en == 1 or (en == 3 and (tn % 2 == 0 or tn >= 28)):
                            nc.scalar.copy(stage[:, esl], ps[:])
                        else:
                            nc.vector.tensor_copy(stage[:, esl], ps[:])
                        if tn >= 30 and en % 2 == 1:
                            # tail: flush each half as soon as it's staged
                            nc.sync.dma_start(
                                o_d[tn * P:(tn + 1) * P,
                                    (en - 1) * IC:(en + 1) * IC],
                                stage[:, (en - 1) * IC:(en + 1) * IC])
                    if tn < 30:
                        nc.sync.dma_start(o_d[tn * P:(tn + 1) * P, :],
                                          stage[:])

                # batch 0 attention
                for h in range(2):
                    vbh = vbh00 if h == 0 else load_vbh(0, 1)
                    for icn in range(NIC):
                        attn_ic(0, h, icn * IC, IC, vbh)
                # batch 1 attention; interleave batch-0 out-projection during
                # h=0 and batch-1 chunks (as their at-slices complete) in h=1
                vbh = load_vbh(1, 0)
                for icn in range(NIC):
                    attn_ic(1, 0, icn * IC, IC, vbh)
                    for k in range(4):
                        ph3_tn(icn * 4 + k)
                # h=1: the last 512-query chunk is split in half so the final
                # out-projection chunks start (and the tail ends) earlier
                vbh = load_vbh(1, 1)
                for icn in range(NIC - 1):
                    attn_ic(1, 1, icn * IC, IC, vbh)
                    if icn > 0:
                        for k in range(4):
                            ph3_tn(16 + (icn - 1) * 4 + k)
                attn_ic(1, 1, 3 * IC, IC // 2, vbh)
                for k in range(4):
                    ph3_tn(16 + 2 * 4 + k)
                attn_ic(1, 1, 3 * IC + IC // 2, IC // 2, vbh)
                for tn in (28, 29, 30, 31):
                    ph3_tn(tn)

    nc.finalize()
    return nc


def _rope_tables():
    """cos/sin tables in [head_dim, token] layout, matching the reference's
    f32 computation (jax on CPU when available)."""
    try:
        import jax
        import jax.numpy as jnp
        cpu = jax.devices("cpu")[0]
        with jax.default_device(cpu):
            inv = 1.0 / (10000.0 ** (
                jnp.arange(0, P, 2, dtype=jnp.float32) / P))
            t = jnp.arange(S, dtype=jnp.float32)
            freqs = jnp.einsum("i,j->ij", t, inv)          # [S, 64]
            emb = jnp.concatenate((freqs, freqs), axis=-1)  # [S, 128]
            cos = np.asarray(jnp.cos(emb)).T                # [128, S]
            sin = np.asarray(jnp.sin(emb)).T
    except Exception:
        inv = 1.0 / (10000.0 ** (np.arange(0, P, 2, dtype=np.float64) / P))
        t = np.arange(S, dtype=np.float64)
        freqs = np.outer(t, inv)
        emb = np.concatenate((freqs, freqs), axis=-1)
        cos = np.cos(emb).T.astype(np.float32)
        sin = np.sin(emb).T.astype(np.float32)

    cos2 = np.ascontiguousarray(np.tile(cos, (1, B)).astype(np.float32))
    sin_z = np.concatenate([sin[0:64], -sin[64:128]], axis=0)
    sin2 = np.ascontiguousarray(np.tile(sin_z, (1, B)).astype(np.float32))
    return cos2, sin2


_NC_CACHE = None


def _rearr_w(w):
    """[DIM, m] -> [P, KC, m] with partition-contiguous rows."""
    m = w.shape[1]
    return np.ascontiguousarray(
        w.reshape(KC, P, m).transpose(1, 0, 2)).astype(np.float32)


def _in_maps(x, Wq, Wk, Wv, Wo):
    # [NTC, P, KC, TC]: xT[n, p, c, t] = x^T[c*P+p, n*TC+t] -- each token
    # chunk is one fully-contiguous 32KB-per-partition DMA
    xT = np.ascontiguousarray(
        x.reshape(TOK, DIM).T.reshape(KC, P, NTC, TC).transpose(2, 1, 0, 3)
    ).astype(np.float32)
    cosT, sinZ = _rope_tables()
    ones = np.ones((P, P), dtype=AVD_NP)
    maps = []
    for c in range(N_CORES):
        cs = slice(c * INNER_C, (c + 1) * INNER_C)
        maps.append({
            "xT": xT,
            "wq": _rearr_w(Wq[:, cs]),
            "wk": _rearr_w(Wk[:, cs]),
            "wv": _rearr_w(Wv[:, cs]),
            "wo": np.ascontiguousarray(
                Wo[cs, :].reshape(2, P, DIM).transpose(1, 0, 2)
            ).astype(np.float16),
            "cosT": cosT,
            "sinZ": sinZ,
            "ones": ones,
        })
    return maps


def kernel(x, Wq, Wk, Wv, Wo):
    global _NC_CACHE
    assert x.shape == (B, S, DIM)
    if _NC_CACHE is None:
        _NC_CACHE = _build()
    in_maps = _in_maps(x, Wq, Wk, Wv, Wo)
    last_err = None
    for attempt in range(3):
        try:
            res = run_bass_kernel_spmd(_NC_CACHE, in_maps,
                                       core_ids=list(range(N_CORES)),
                                       trace=False)
            break
        except Exception as e:  # transient NRT faults: retry
            last_err = e
    else:
        raise last_err
    out = res.results[0]["o_part"].astype(np.float32)
    for c in range(1, N_CORES):
        out += res.results[c]["o_part"].astype(np.float32)
    return out.astype(np.float32).reshape(B, S, DIM)



# revision 3
# speedup vs baseline: 1.1370x; 1.1370x over previous
"""Trainium2 Bass kernel for nn_Attention_11287174054323.

Full attention layer: QKV projections + RoPE + softmax attention + output
projection.  B=2, S=2048, DIM=2048, 16 heads x 128 head_dim, fp32 I/O.

Sharding: tensor-parallel over heads across 8 NeuronCores (2 heads/core).
Each core computes q/k/v projections for its head slice, full attention for
its heads, and a partial output projection (row slice of Wo); the host sums
the 8 partials.

Per-core layout strategy:
  - x is passed pre-transposed (xT [DIM, B*S]) so projections can contract
    over DIM on the partition axis.
  - Q^T/K^T are produced in [head_dim, token] layout; RoPE is fused into the
    PSUM eviction (rotate-half via cross-partition-write multiplies).
  - Scores are computed transposed (S^T = K @ Q^T), softmax-normalization is
    deferred past the A^T = V^T @ exp(S^T) accumulation (linearity); the
    denominator is accumulated with DVE adds (two independent accumulators
    to halve the serial chain), reduced+broadcast with ones-matmuls in PSUM,
    inverted with reciprocal_approx_fast.
  - Projections/scores run in float32r; the exp/AV group and the
    out-projection operands are fp16.
  - Weights/x are passed pre-rearranged to partition-contiguous layouts.
  - PE warm-up: a few matmuls on a zeroed tile at t=0 start the Tensor
    engine p-state ramp while the first weight/x DMAs are in flight.
  - Phase 1 streams x on the ACT engine's DMA queue in parallel with
    weights on SP; wq/wk chunk DMAs are interleaved so both Q and K
    matmuls are enabled early in the first token chunk.
  - Out-projection 128-token groups (ph3) are spread across the attention
    i-chunks as soon as their at-slices are complete (starting in batch-0
    h=1), instead of being concentrated at the end; PSUM evictions are
    split into two 256-wide halves issued on ACT and DVE concurrently.
  - The partial output is staged to fp16 and summed on the host in fp32.
"""

import sys

sys.path.insert(0, "/opt/trn_rl_repo")

import numpy as np

import concourse.tile as tile
import concourse.mybir as mybir
from concourse import bacc
from concourse.bass_utils import run_bass_kernel_spmd

P = 128
B, S, DIM = 2, 2048, 2048
TOK = B * S                     # 4096 tokens
HEADS_PER_CORE = 2
INNER_C = HEADS_PER_CORE * P    # 256 per-core inner dim
KC = DIM // P                   # 16 contraction chunks
TC = 512                        # phase-1 token chunk
NTC = TOK // TC                 # 8
IC = 512                        # attention i-chunk (queries)
NIC = S // IC                   # 4 per (batch, head)
NJC = S // P                    # 16 key chunks per (batch, head)
SCALE = float(P) ** -0.5

F32 = mybir.dt.float32
F16 = mybir.dt.float16
MM = mybir.dt.float32r          # matmul dtype for projections/scores
AVD = mybir.dt.float16          # dtype of the exp/AV group (e, acc, v, ones)
AVD_NP = np.float16

N_CORES = 8
Mul = mybir.AluOpType.mult
N_WARM = 10                     # PE warm-up matmuls


def _build():
    nc = bacc.Bacc("TRN2", target_bir_lowering=False)

    # weights/activations arrive pre-rearranged from the host so every DMA
    # is contiguous per partition (strided gathers cost 10-16us to issue)
    xT_d = nc.dram_tensor("xT", [NTC, P, KC, TC], MM, kind="ExternalInput")
    wq_d = nc.dram_tensor("wq", [P, KC, INNER_C], MM, kind="ExternalInput")
    wk_d = nc.dram_tensor("wk", [P, KC, INNER_C], MM, kind="ExternalInput")
    wv_d = nc.dram_tensor("wv", [P, KC, INNER_C], MM, kind="ExternalInput")
    wo_d = nc.dram_tensor("wo", [P, 2, DIM], F16, kind="ExternalInput")
    cos_d = nc.dram_tensor("cosT", [P, TOK], F32, kind="ExternalInput")
    sin_d = nc.dram_tensor("sinZ", [P, TOK], F32, kind="ExternalInput")
    ones_d = nc.dram_tensor("ones", [P, P], AVD, kind="ExternalInput")
    o_d = nc.dram_tensor("o_part", [TOK, DIM], F16, kind="ExternalOutput")


    with tile.TileContext(nc) as tc:
        with tc.tile_pool(name="persist", bufs=1) as persist, \
             tc.tile_pool(name="dram", bufs=1, space="DRAM") as dram:
            qt = [persist.tile([P, TOK], F16, tag=f"qt{h}", name=f"qt{h}")
                  for h in range(2)]
            kt = [persist.tile([P, TOK], F16, tag=f"kt{h}", name=f"kt{h}")
                  for h in range(2)]
            ones_t = persist.tile([P, P], AVD, tag="ones")
            v_dram = dram.tile([TOK, INNER_C], AVD)

            # ---------------- Phase 1: QKV projections + RoPE ----------
            with tc.tile_pool(name="w1", bufs=1) as wpool, \
                 tc.tile_pool(name="x1", bufs=2) as xpool, \
                 tc.tile_pool(name="ev1", bufs=2) as evpool, \
                 tc.tile_pool(name="ps1", bufs=1, space="PSUM") as psum1:
                # PE warm-up: matmuls on a zeroed tile, results discarded.
                # Starts the Tensor-engine p-state ramp (~3us of continuous
                # execution to reach max clock) while the first weight/x
                # DMAs are still in flight.
                wz = evpool.tile([P, TC], F16, tag="warm_z")
                nc.vector.memzero(wz[:])
                wps = psum1.tile([P, TC], F32, tag="ps_q0")
                for _ in range(N_WARM):
                    nc.tensor.matmul(wps[:], wz[:, 0:P], wz[:],
                                     start=True, stop=True)

                wq_t = wpool.tile([P, KC, INNER_C], MM, tag="wq")
                wk_t = wpool.tile([P, KC, INNER_C], MM, tag="wk")
                wv_t = wpool.tile([P, KC, INNER_C], MM, tag="wv")
                # Two parallel DMA queues in phase 1: SP streams weights
                # (wq/wk interleaved chunk-wise so K matmuls start early),
                # the (otherwise idle) ACT engine queue streams x chunks.
                nc.sync.dma_start(wq_t[:, 0:1, :], wq_d[:, 0:1, :])
                xt0 = xpool.tile([P, KC, TC], MM, tag="xt")
                nc.scalar.dma_start(xt0[:, 0:1, :], xT_d[0, :, 0:1, :])
                nc.sync.dma_start(wq_t[:, 1:4, :], wq_d[:, 1:4, :])
                nc.scalar.dma_start(xt0[:, 1:2, :], xT_d[0, :, 1:2, :])
                nc.sync.dma_start(wk_t[:, 0:4, :], wk_d[:, 0:4, :])
                nc.scalar.dma_start(xt0[:, 2:4, :], xT_d[0, :, 2:4, :])
                nc.sync.dma_start(wq_t[:, 4:KC, :], wq_d[:, 4:KC, :])
                nc.scalar.dma_start(xt0[:, 4:KC // 2, :],
                                    xT_d[0, :, 4:KC // 2, :])
                nc.sync.dma_start(wk_t[:, 4:KC, :], wk_d[:, 4:KC, :])
                nc.scalar.dma_start(xt0[:, KC // 2:KC, :],
                                    xT_d[0, :, KC // 2:KC, :])
                cos0 = evpool.tile([P, TC], F32, tag="cos")
                sin0 = evpool.tile([P, TC], F32, tag="sin")
                nc.scalar.dma_start(cos0[:], cos_d[:, 0:TC])
                nc.scalar.dma_start(sin0[:], sin_d[:, 0:TC])
                nc.sync.dma_start(wv_t[:], wv_d[:])
                nc.sync.dma_start(ones_t[:], ones_d[:])

                for tcn in range(NTC):
                    tsl = slice(tcn * TC, (tcn + 1) * TC)
                    if tcn == 0:
                        xt, cos_t, sin_t = xt0, cos0, sin0
                    else:
                        xt = xpool.tile([P, KC, TC], MM, tag="xt")
                        nc.scalar.dma_start(xt[:], xT_d[tcn])
                        cos_t = evpool.tile([P, TC], F32, tag="cos")
                        sin_t = evpool.tile([P, TC], F32, tag="sin")
                        nc.sync.dma_start(cos_t[:], cos_d[:, tsl])
                        nc.sync.dma_start(sin_t[:], sin_d[:, tsl])

                    # Q^T / K^T chunks with fused RoPE eviction
                    for wt, dsts, nm in ((wq_t, qt, "q"), (wk_t, kt, "k")):
                        for m in range(2):
                            ps = psum1.tile([P, TC], F32, tag=f"ps_{nm}{m}")
                            for kc in range(KC):
                                nc.tensor.matmul(
                                    ps[:], wt[:, kc, m * P:(m + 1) * P],
                                    xt[:, kc, :],
                                    start=(kc == 0), stop=(kc == KC - 1))
                            # rope: dst = ps*cos + rotate_half(ps)*sin
                            tcos = evpool.tile([P, TC], F32, tag="tcos")
                            nc.vector.tensor_mul(tcos[:], ps[:], cos_t[:])
                            tsin = evpool.tile([P, TC], F32, tag="tsin")
                            nc.vector.scalar_tensor_tensor(
                                tsin[0:64, :], ps[64:128, :], 1.0,
                                sin_t[64:128, :], Mul, Mul)
                            nc.vector.scalar_tensor_tensor(
                                tsin[64:128, :], ps[0:64, :], 1.0,
                                sin_t[0:64, :], Mul, Mul)
                            nc.vector.tensor_add(dsts[m][:, tsl], tcos[:],
                                                 tsin[:])

                    # V chunks (tokens on partitions) -> DRAM scratch
                    for m in range(TC // P):
                        ps = psum1.tile([P, INNER_C], F32, tag=f"ps_v{m}")
                        for kc in range(KC):
                            nc.tensor.matmul(
                                ps[:], xt[:, kc, m * P:(m + 1) * P],
                                wv_t[:, kc, :],
                                start=(kc == 0), stop=(kc == KC - 1))
                        vst = evpool.tile([P, INNER_C], AVD, tag="vst")
                        nc.scalar.copy(vst[:], ps[:])
                        r0 = tcn * TC + m * P
                        nc.sync.dma_start(v_dram[r0:r0 + P, :], vst[:])

            # ---------- Phases 2+3: attention + output projection ------
            with tc.tile_pool(name="at", bufs=1) as atpool, \
                 tc.tile_pool(name="vbh", bufs=2) as vpool, \
                 tc.tile_pool(name="e2", bufs=4) as epool, \
                 tc.tile_pool(name="sm2", bufs=2) as smpool, \
                 tc.tile_pool(name="p3", bufs=1) as p3pool, \
                 tc.tile_pool(name="st3", bufs=3) as stpool, \
                 tc.tile_pool(name="ps2", bufs=1, space="PSUM") as psum2, \
                 tc.tile_pool(name="ps3", bufs=2, space="PSUM") as psum3:
                at = [atpool.tile([P, TOK], F16, tag=f"at{h}", name=f"at{h}")
                      for h in range(2)]

                def load_vbh(b, h):
                    boff = b * S
                    vbh = vpool.tile([P, NJC, P], AVD, tag="vbh")
                    nc.sync.dma_start(
                        vbh[:],
                        v_dram[boff:boff + S, h * P:(h + 1) * P]
                        .rearrange("(c p) d -> p c d", p=P))
                    return vbh

                # vbh(0,0) before the (large) wo load: the first AV matmul
                # needs it early, while wo is only needed at the first
                # ph3_tn.
                vbh00 = load_vbh(0, 0)
                wo_t = p3pool.tile([P, 2, DIM], F16, tag="wo")
                nc.sync.dma_start(wo_t[:], wo_d[:])

                def attn_ic(b, h, i0, iw, vbh):
                    """One iw-query chunk (batch offset i0) of attention for
                    (batch, head)."""
                    boff = b * S
                    isl = slice(boff + i0, boff + i0 + iw)
                    ps_at = psum2.tile([P, IC], F32, tag="ps_at", bufs=2)
                    acc = smpool.tile([P, 2 * IC], AVD, tag="acc")
                    acc2 = smpool.tile([P, 2 * IC], AVD, tag="acc2")
                    # software-pipelined (2x S) -> exp -> (A, colsum) chain
                    NJP = NJC // 2
                    es = [None] * NJP

                    def sp_step(p):
                        ps_s = psum2.tile([P, 2 * IC], F32, tag="ps_s", bufs=2)
                        for half in range(2):
                            jc = 2 * p + half
                            jsl = slice(boff + jc * P, boff + (jc + 1) * P)
                            nc.tensor.matmul(
                                ps_s[:, half * iw:(half + 1) * iw],
                                kt[h][:, jsl], qt[h][:, isl],
                                start=True, stop=True)
                        e = epool.tile([P, 2 * IC], AVD, tag="e")
                        nc.scalar.activation(
                            e[:, 0:2 * iw], ps_s[:, 0:2 * iw],
                            mybir.ActivationFunctionType.Exp,
                            scale=SCALE)
                        es[p] = e

                    def a_step(p):
                        e = es[p]
                        for half in range(2):
                            jc = 2 * p + half
                            eh = e[:, half * iw:(half + 1) * iw]
                            nc.tensor.matmul(ps_at[:, 0:iw], vbh[:, jc, :], eh,
                                             start=(jc == 0),
                                             stop=(jc == NJC - 1))
                        # two independent accumulators halve the serial
                        # dependency chain of the denominator adds
                        dst = acc if p % 2 == 0 else acc2
                        if p < 2:
                            nc.vector.tensor_copy(dst[:, 0:2 * iw],
                                                  e[:, 0:2 * iw])
                        else:
                            nc.vector.tensor_add(dst[:, 0:2 * iw],
                                                 dst[:, 0:2 * iw],
                                                 e[:, 0:2 * iw])

                    sp_step(0)
                    for p in range(NJP):
                        if p + 1 < NJP:
                            sp_step(p + 1)
                        a_step(p)
                    nc.vector.tensor_add(acc[:, 0:2 * iw], acc[:, 0:2 * iw],
                                         acc2[:, 0:2 * iw])

                    ps_bc = psum3.tile([P, IC], F32, tag="ps_o")
                    nc.tensor.matmul(ps_bc[:, 0:iw], ones_t[:], acc[:, 0:iw],
                                     start=True, stop=False)
                    nc.tensor.matmul(ps_bc[:, 0:iw], ones_t[:],
                                     acc[:, iw:2 * iw],
                                     start=False, stop=True)
                    recip = smpool.tile([P, IC], F32, tag="recip")
                    nc.vector.reciprocal_approx_fast(recip[:, 0:iw],
                                                     ps_bc[:, 0:iw])
                    nc.vector.tensor_mul(at[h][:, isl], ps_at[:, 0:iw],
                                         recip[:, 0:iw])

                def ph3_tn(tn):
                    """One 128-token chunk of the output projection.
                    Each [128, 512] PSUM eviction is split into two 256-wide
                    halves issued on ACT and DVE concurrently (halves the
                    eviction latency that gates PSUM-bank reuse)."""
                    stage = stpool.tile([P, DIM], F16, tag="stage")
                    HB = IC // 2
                    for en in range(DIM // IC):
                        ps = psum3.tile([P, IC], F32, tag="ps_o")
                        esl = slice(en * IC, (en + 1) * IC)
                        for h in range(2):
                            nc.tensor.matmul(
                                ps[:], at[h][:, tn * P:(tn + 1) * P],
                                wo_t[:, h, esl],
                                start=(h == 0), stop=(h == 1))
                        lo = slice(en * IC, en * IC + HB)
                        hi = slice(en * IC + HB, (en + 1) * IC)
                        nc.scalar.copy(stage[:, lo], ps[:, 0:HB])
                        nc.vector.tensor_copy(stage[:, hi], ps[:, HB:IC])
                        if tn >= 30 and en % 2 == 1:
                            # tail: flush each half as soon as it's staged
                            nc.sync.dma_start(
                                o_d[tn * P:(tn + 1) * P,
                                    (en - 1) * IC:(en + 1) * IC],
                                stage[:, (en - 1) * IC:(en + 1) * IC])
                    if tn < 30:
                        nc.sync.dma_start(o_d[tn * P:(tn + 1) * P, :],
                                          stage[:])

                # ---- schedule ----
                # batch 0, head 0: no out-projection available yet
                for icn in range(NIC):
                    attn_ic(0, 0, icn * IC, IC, vbh00)
                # batch 0, head 1: tn groups 0..5 become available as the
                # at-slices complete (tn k needs both heads for tokens
                # [k*128,(k+1)*128), i.e. b0h1 ic >= k//4 done)
                vbh = load_vbh(0, 1)
                for icn in range(NIC):
                    attn_ic(0, 1, icn * IC, IC, vbh)
                    if icn >= 1:
                        for k in range(2):
                            ph3_tn((icn - 1) * 2 + k)
                # batch 1, head 0: finish batch-0 out-projection (tn 6..15)
                vbh = load_vbh(1, 0)
                b0_rest = list(range(6, 16))
                for icn in range(NIC):
                    attn_ic(1, 0, icn * IC, IC, vbh)
                    take = 3 if icn < 2 else 2
                    for _ in range(take):
                        if b0_rest:
                            ph3_tn(b0_rest.pop(0))
                for tn in b0_rest:
                    ph3_tn(tn)
                # batch 1, head 1: batch-1 tn groups become available per ic;
                # the last 512-query chunk is split in half so the final
                # out-projection chunks start (and the tail ends) earlier
                vbh = load_vbh(1, 1)
                for icn in range(NIC - 1):
                    attn_ic(1, 1, icn * IC, IC, vbh)
                    if icn > 0:
                        for k in range(4):
                            ph3_tn(16 + (icn - 1) * 4 + k)
                attn_ic(1, 1, 3 * IC, IC // 2, vbh)
                for k in range(4):
                    ph3_tn(16 + 2 * 4 + k)
                attn_ic(1, 1, 3 * IC + IC // 2, IC // 2, vbh)
                for tn in (28, 29, 30, 31):
                    ph3_tn(tn)

    nc.finalize()
    return nc


def _rope_tables():
    """cos/sin tables in [head_dim, token] layout, matching the reference's
    f32 computation (jax on CPU when available)."""
    try:
        import jax
        import jax.numpy as jnp
        cpu = jax.devices("cpu")[0]
        with jax.default_device(cpu):
            inv = 1.0 / (10000.0 ** (
                jnp.arange(0, P, 2, dtype=jnp.float32) / P))
            t = jnp.arange(S, dtype=jnp.float32)
            freqs = jnp.einsum("i,j->ij", t, inv)          # [S, 64]
            emb = jnp.concatenate((freqs, freqs), axis=-1)  # [S, 128]
            cos = np.asarray(jnp.cos(emb)).T                # [128, S]
            sin = np.asarray(jnp.sin(emb)).T
    except Exception:
        inv = 1.0 / (10000.0 ** (np.arange(0, P, 2, dtype=np.float64) / P))
        t = np.arange(S, dtype=np.float64)
        freqs = np.outer(t, inv)
        emb = np.concatenate((freqs, freqs), axis=-1)
        cos = np.cos(emb).T.astype(np.float32)
        sin = np.sin(emb).T.astype(np.float32)

    cos2 = np.ascontiguousarray(np.tile(cos, (1, B)).astype(np.float32))
    sin_z = np.concatenate([sin[0:64], -sin[64:128]], axis=0)
    sin2 = np.ascontiguousarray(np.tile(sin_z, (1, B)).astype(np.float32))
    return cos2, sin2


_NC_CACHE = None


def _rearr_w(w):
    """[DIM, m] -> [P, KC, m] with partition-contiguous rows."""
    m = w.shape[1]
    return np.ascontiguousarray(
        w.reshape(KC, P, m).transpose(1, 0, 2)).astype(np.float32)


def _in_maps(x, Wq, Wk, Wv, Wo):
    # [NTC, P, KC, TC]: xT[n, p, c, t] = x^T[c*P+p, n*TC+t] -- each token
    # chunk is one fully-contiguous 32KB-per-partition DMA
    xT = np.ascontiguousarray(
        x.reshape(TOK, DIM).T.reshape(KC, P, NTC, TC).transpose(2, 1, 0, 3)
    ).astype(np.float32)
    cosT, sinZ = _rope_tables()
    ones = np.ones((P, P), dtype=AVD_NP)
    maps = []
    for c in range(N_CORES):
        cs = slice(c * INNER_C, (c + 1) * INNER_C)
        maps.append({
            "xT": xT,
            "wq": _rearr_w(Wq[:, cs]),
            "wk": _rearr_w(Wk[:, cs]),
            "wv": _rearr_w(Wv[:, cs]),
            "wo": np.ascontiguousarray(
                Wo[cs, :].reshape(2, P, DIM).transpose(1, 0, 2)
            ).astype(np.float16),
            "cosT": cosT,
            "sinZ": sinZ,
            "ones": ones,
        })
    return maps


def kernel(x, Wq, Wk, Wv, Wo):
    global _NC_CACHE
    assert x.shape == (B, S, DIM)
    if _NC_CACHE is None:
        _NC_CACHE = _build()
    in_maps = _in_maps(x, Wq, Wk, Wv, Wo)
    last_err = None
    for attempt in range(3):
        try:
            res = run_bass_kernel_spmd(_NC_CACHE, in_maps,
                                       core_ids=list(range(N_CORES)),
                                       trace=False)
            break
        except Exception as e:  # transient NRT faults: retry
            last_err = e
    else:
        raise last_err
    out = res.results[0]["o_part"].astype(np.float32)
    for c in range(1, N_CORES):
        out += res.results[c]["o_part"].astype(np.float32)
    return out.astype(np.float32).reshape(B, S, DIM)


# revision 10
# speedup vs baseline: 1.1644x; 1.0241x over previous
"""Trainium2 Bass kernel for nn_Attention_11287174054323.

Full attention layer: QKV projections + RoPE + softmax attention + output
projection.  B=2, S=2048, DIM=2048, 16 heads x 128 head_dim, fp32 I/O.

Sharding: tensor-parallel over heads across 8 NeuronCores (2 heads/core).
Each core computes q/k/v projections for its head slice, full attention for
its heads, and a partial output projection (row slice of Wo); the host sums
the 8 partials.

Per-core layout strategy:
  - x is passed pre-transposed (xT [DIM, B*S]) so projections can contract
    over DIM on the partition axis.
  - Q^T/K^T are produced in [head_dim, token] layout; RoPE is fused into the
    PSUM eviction (rotate-half via cross-partition-write multiplies).
  - Scores are computed transposed (S^T = K @ Q^T), softmax-normalization is
    deferred past the A^T = V^T @ exp(S^T) accumulation (linearity); the
    denominator is accumulated with DVE adds (two independent accumulators
    to halve the serial chain), reduced+broadcast with ones-matmuls in PSUM,
    inverted with reciprocal_approx_fast.
  - Projections/scores run in float32r; the exp/AV group and the
    out-projection operands are fp16.
  - Weights/x are passed pre-rearranged to partition-contiguous layouts.
  - PE warm-up: a few matmuls on a zeroed tile at t=0 start the Tensor
    engine p-state ramp while the first weight/x DMAs are in flight.
  - Phase 1 streams x on the ACT engine's DMA queue in parallel with
    weights on SP; wq/wk chunk DMAs are interleaved so both Q and K
    matmuls are enabled early in the first token chunk.
  - Out-projection 128-token groups (ph3) are spread across the attention
    i-chunks as soon as their at-slices are complete (starting in batch-0
    h=1), instead of being concentrated at the end; PSUM evictions are
    split into two 256-wide halves issued on ACT and DVE concurrently.
  - The partial output is staged to fp16 and summed on the host in fp32.
"""

import sys

sys.path.insert(0, "/opt/trn_rl_repo")

import numpy as np

import concourse.tile as tile
import concourse.mybir as mybir
from concourse import bacc
from concourse.bass_utils import run_bass_kernel_spmd

P = 128
B, S, DIM = 2, 2048, 2048
TOK = B * S                     # 4096 tokens
HEADS_PER_CORE = 2
INNER_C = HEADS_PER_CORE * P    # 256 per-core inner dim
KC = DIM // P                   # 16 contraction chunks
TC = 512                        # phase-1 token chunk
NTC = TOK // TC                 # 8
IC = 512                        # attention i-chunk (queries)
NIC = S // IC                   # 4 per (batch, head)
NJC = S // P                    # 16 key chunks per (batch, head)
SCALE = float(P) ** -0.5

F32 = mybir.dt.float32
F16 = mybir.dt.float16
MM = mybir.dt.float16           # matmul dtype for projections (x, wq/wk/wv)
MM_NP = np.float16
AVD = mybir.dt.float16          # dtype of the exp/AV group (e, acc, v, ones)
AVD_NP = np.float16

N_CORES = 8
Mul = mybir.AluOpType.mult
N_WARM = 10                     # PE warm-up matmuls


def _build():
    nc = bacc.Bacc("TRN2", target_bir_lowering=False)

    # weights/activations arrive pre-rearranged from the host so every DMA
    # is contiguous per partition (strided gathers cost 10-16us to issue)
    xT_d = nc.dram_tensor("xT", [NTC, P, KC, TC], MM, kind="ExternalInput")
    wq_d = nc.dram_tensor("wq", [P, KC, INNER_C], MM, kind="ExternalInput")
    wk_d = nc.dram_tensor("wk", [P, KC, INNER_C], MM, kind="ExternalInput")
    wv_d = nc.dram_tensor("wv", [P, KC, INNER_C], MM, kind="ExternalInput")
    wo_d = nc.dram_tensor("wo", [P, 2, DIM], F16, kind="ExternalInput")
    cos_d = nc.dram_tensor("cosT", [P, TOK], F32, kind="ExternalInput")
    sin_d = nc.dram_tensor("sinZ", [P, TOK], F32, kind="ExternalInput")
    ones_d = nc.dram_tensor("ones", [P, P], AVD, kind="ExternalInput")
    o_d = nc.dram_tensor("o_part", [TOK, DIM], F16, kind="ExternalOutput")


    with tile.TileContext(nc) as tc:
        with tc.tile_pool(name="persist", bufs=1) as persist, \
             tc.tile_pool(name="dram", bufs=1, space="DRAM") as dram:
            qt = [persist.tile([P, TOK], F16, tag=f"qt{h}", name=f"qt{h}")
                  for h in range(2)]
            kt = [persist.tile([P, TOK], F16, tag=f"kt{h}", name=f"kt{h}")
                  for h in range(2)]
            ones_t = persist.tile([P, P], AVD, tag="ones")
            # first-attention operands live in the persistent pool so their
            # DMAs can be issued mid-phase-1 (prefetch)
            vbh00 = persist.tile([P, NJC, P], AVD, tag="vbh00")
            wo_t = persist.tile([P, 2, DIM], F16, tag="wo")
            v_dram = dram.tile([TOK, INNER_C], AVD)

            # ---------------- Phase 1: QKV projections + RoPE ----------
            with tc.tile_pool(name="w1", bufs=1) as wpool, \
                 tc.tile_pool(name="x1", bufs=2) as xpool, \
                 tc.tile_pool(name="ev1", bufs=2) as evpool, \
                 tc.tile_pool(name="ps1", bufs=1, space="PSUM") as psum1:
                # PE warm-up: matmuls on a zeroed tile, results discarded.
                # Starts the Tensor-engine p-state ramp (~3us of continuous
                # execution to reach max clock) while the first weight/x
                # DMAs are still in flight.
                wz = evpool.tile([P, TC], F16, tag="warm_z")
                nc.vector.memzero(wz[:])
                wps = psum1.tile([P, TC], F32, tag="ps_q0")
                for _ in range(N_WARM):
                    nc.tensor.matmul(wps[:], wz[:, 0:P], wz[:],
                                     start=True, stop=True)

                wq_t = wpool.tile([P, KC, INNER_C], MM, tag="wq")
                wk_t = wpool.tile([P, KC, INNER_C], MM, tag="wk")
                wv_t = wpool.tile([P, KC, INNER_C], MM, tag="wv")
                # Two parallel DMA queues in phase 1: SP streams weights
                # (wq/wk interleaved chunk-wise so K matmuls start early),
                # the (otherwise idle) ACT engine queue streams x chunks.
                nc.sync.dma_start(wq_t[:, 0:1, :], wq_d[:, 0:1, :])
                xt0 = xpool.tile([P, KC, TC], MM, tag="xt")
                nc.scalar.dma_start(xt0[:, 0:1, :], xT_d[0, :, 0:1, :])
                nc.sync.dma_start(wq_t[:, 1:4, :], wq_d[:, 1:4, :])
                nc.scalar.dma_start(xt0[:, 1:2, :], xT_d[0, :, 1:2, :])
                nc.sync.dma_start(wk_t[:, 0:4, :], wk_d[:, 0:4, :])
                nc.scalar.dma_start(xt0[:, 2:4, :], xT_d[0, :, 2:4, :])
                nc.sync.dma_start(wq_t[:, 4:KC, :], wq_d[:, 4:KC, :])
                nc.scalar.dma_start(xt0[:, 4:KC // 2, :],
                                    xT_d[0, :, 4:KC // 2, :])
                nc.sync.dma_start(wk_t[:, 4:KC, :], wk_d[:, 4:KC, :])
                nc.scalar.dma_start(xt0[:, KC // 2:KC, :],
                                    xT_d[0, :, KC // 2:KC, :])
                cos0 = evpool.tile([P, TC], F32, tag="cos")
                sin0 = evpool.tile([P, TC], F32, tag="sin")
                nc.sync.dma_start(wv_t[:, 0:KC // 2, :], wv_d[:, 0:KC // 2, :])
                nc.scalar.dma_start(cos0[:], cos_d[:, 0:TC])
                nc.scalar.dma_start(sin0[:], sin_d[:, 0:TC])
                nc.sync.dma_start(wv_t[:, KC // 2:KC, :],
                                  wv_d[:, KC // 2:KC, :])
                nc.sync.dma_start(ones_t[:], ones_d[:])

                for tcn in range(NTC):
                    if tcn == 4:
                        # Prefetch the first attention operands while the
                        # back half of phase 1 runs: vbh(0,0) only needs
                        # v_dram rows 0..2047 (written by tcn 0-3), and wo
                        # is needed at the first out-projection chunk.
                        nc.sync.dma_start(
                            vbh00[:],
                            v_dram[0:S, 0:P]
                            .rearrange("(c p) d -> p c d", p=P))
                        nc.sync.dma_start(wo_t[:], wo_d[:])
                    tsl = slice(tcn * TC, (tcn + 1) * TC)
                    if tcn == 0:
                        xt, cos_t, sin_t = xt0, cos0, sin0
                    else:
                        xt = xpool.tile([P, KC, TC], MM, tag="xt")
                        nc.scalar.dma_start(xt[:], xT_d[tcn])
                        cos_t = evpool.tile([P, TC], F32, tag="cos")
                        sin_t = evpool.tile([P, TC], F32, tag="sin")
                        nc.sync.dma_start(cos_t[:], cos_d[:, tsl])
                        nc.sync.dma_start(sin_t[:], sin_d[:, tsl])

                    # Q^T / K^T chunks with fused RoPE eviction
                    for wt, dsts, nm in ((wq_t, qt, "q"), (wk_t, kt, "k")):
                        for m in range(2):
                            ps = psum1.tile([P, TC], F32, tag=f"ps_{nm}{m}")
                            for kc in range(KC):
                                nc.tensor.matmul(
                                    ps[:], wt[:, kc, m * P:(m + 1) * P],
                                    xt[:, kc, :],
                                    start=(kc == 0), stop=(kc == KC - 1))
                            # rope: dst = ps*cos + rotate_half(ps)*sin
                            tcos = evpool.tile([P, TC], F32, tag="tcos")
                            nc.vector.tensor_mul(tcos[:], ps[:], cos_t[:])
                            tsin = evpool.tile([P, TC], F32, tag="tsin")
                            nc.vector.scalar_tensor_tensor(
                                tsin[0:64, :], ps[64:128, :], 1.0,
                                sin_t[64:128, :], Mul, Mul)
                            nc.vector.scalar_tensor_tensor(
                                tsin[64:128, :], ps[0:64, :], 1.0,
                                sin_t[0:64, :], Mul, Mul)
                            nc.vector.tensor_add(dsts[m][:, tsl], tcos[:],
                                                 tsin[:])

                    # V chunks (tokens on partitions) -> DRAM scratch
                    for m in range(TC // P):
                        ps = psum1.tile([P, INNER_C], F32, tag=f"ps_v{m}")
                        for kc in range(KC):
                            nc.tensor.matmul(
                                ps[:], xt[:, kc, m * P:(m + 1) * P],
                                wv_t[:, kc, :],
                                start=(kc == 0), stop=(kc == KC - 1))
                        vst = evpool.tile([P, INNER_C], AVD, tag="vst")
                        nc.scalar.copy(vst[:], ps[:])
                        r0 = tcn * TC + m * P
                        nc.sync.dma_start(v_dram[r0:r0 + P, :], vst[:])

            # ---------- Phases 2+3: attention + output projection ------
            with tc.tile_pool(name="at", bufs=1) as atpool, \
                 tc.tile_pool(name="vbh", bufs=2) as vpool, \
                 tc.tile_pool(name="e2", bufs=4) as epool, \
                 tc.tile_pool(name="sm2", bufs=2) as smpool, \
                 tc.tile_pool(name="st3", bufs=5) as stpool, \
                 tc.tile_pool(name="ps2", bufs=1, space="PSUM") as psum2, \
                 tc.tile_pool(name="ps3", bufs=2, space="PSUM") as psum3:
                at = [atpool.tile([P, TOK], F16, tag=f"at{h}", name=f"at{h}")
                      for h in range(2)]

                def load_vbh(b, h):
                    boff = b * S
                    vbh = vpool.tile([P, NJC, P], AVD, tag="vbh")
                    nc.sync.dma_start(
                        vbh[:],
                        v_dram[boff:boff + S, h * P:(h + 1) * P]
                        .rearrange("(c p) d -> p c d", p=P))
                    return vbh

                def attn_ic(b, h, i0, iw, vbh):
                    """One iw-query chunk (batch offset i0) of attention for
                    (batch, head)."""
                    boff = b * S
                    isl = slice(boff + i0, boff + i0 + iw)
                    ps_at = psum2.tile([P, IC], F32, tag="ps_at", bufs=2)
                    acc = smpool.tile([P, 2 * IC], AVD, tag="acc")
                    acc2 = smpool.tile([P, 2 * IC], AVD, tag="acc2")
                    # software-pipelined (2x S) -> exp -> (A, colsum) chain
                    NJP = NJC // 2
                    es = [None] * NJP

                    def sp_step(p):
                        ps_s = psum2.tile([P, 2 * IC], F32, tag="ps_s", bufs=2)
                        for half in range(2):
                            jc = 2 * p + half
                            jsl = slice(boff + jc * P, boff + (jc + 1) * P)
                            nc.tensor.matmul(
                                ps_s[:, half * iw:(half + 1) * iw],
                                kt[h][:, jsl], qt[h][:, isl],
                                start=True, stop=True)
                        e = epool.tile([P, 2 * IC], AVD, tag="e")
                        nc.scalar.activation(
                            e[:, 0:2 * iw], ps_s[:, 0:2 * iw],
                            mybir.ActivationFunctionType.Exp,
                            scale=SCALE)
                        es[p] = e

                    def a_step(p):
                        e = es[p]
                        for half in range(2):
                            jc = 2 * p + half
                            eh = e[:, half * iw:(half + 1) * iw]
                            nc.tensor.matmul(ps_at[:, 0:iw], vbh[:, jc, :], eh,
                                             start=(jc == 0),
                                             stop=(jc == NJC - 1))
                        # two independent accumulators halve the serial
                        # dependency chain of the denominator adds
                        dst = acc if p % 2 == 0 else acc2
                        if p < 2:
                            nc.vector.tensor_copy(dst[:, 0:2 * iw],
                                                  e[:, 0:2 * iw])
                        else:
                            nc.vector.tensor_add(dst[:, 0:2 * iw],
                                                 dst[:, 0:2 * iw],
                                                 e[:, 0:2 * iw])

                    sp_step(0)
                    for p in range(NJP):
                        if p + 1 < NJP:
                            sp_step(p + 1)
                        a_step(p)
                    nc.vector.tensor_add(acc[:, 0:2 * iw], acc[:, 0:2 * iw],
                                         acc2[:, 0:2 * iw])

                    ps_bc = psum3.tile([P, IC], F32, tag="ps_o")
                    nc.tensor.matmul(ps_bc[:, 0:iw], ones_t[:], acc[:, 0:iw],
                                     start=True, stop=False)
                    nc.tensor.matmul(ps_bc[:, 0:iw], ones_t[:],
                                     acc[:, iw:2 * iw],
                                     start=False, stop=True)
                    recip = smpool.tile([P, IC], F32, tag="recip")
                    nc.vector.reciprocal_approx_fast(recip[:, 0:iw],
                                                     ps_bc[:, 0:iw])
                    nc.vector.tensor_mul(at[h][:, isl], ps_at[:, 0:iw],
                                         recip[:, 0:iw])

                def ph3_tn(tn):
                    """One 128-token chunk of the output projection.
                    Each [128, 512] PSUM eviction is split into two 256-wide
                    halves issued on ACT and DVE concurrently (halves the
                    eviction latency that gates PSUM-bank reuse)."""
                    stage = stpool.tile([P, DIM], F16, tag="stage")
                    HB = IC // 2
                    for en in range(DIM // IC):
                        ps = psum3.tile([P, IC], F32, tag="ps_o")
                        esl = slice(en * IC, (en + 1) * IC)
                        for h in range(2):
                            nc.tensor.matmul(
                                ps[:], at[h][:, tn * P:(tn + 1) * P],
                                wo_t[:, h, esl],
                                start=(h == 0), stop=(h == 1))
                        lo = slice(en * IC, en * IC + HB)
                        hi = slice(en * IC + HB, (en + 1) * IC)
                        nc.scalar.copy(stage[:, lo], ps[:, 0:HB])
                        nc.vector.tensor_copy(stage[:, hi], ps[:, HB:IC])
                        if en % 2 == 1:
                            # flush each 1024-col half as soon as it is
                            # staged, alternating DMA queues
                            q = nc.sync if en == 1 else nc.scalar
                            q.dma_start(
                                o_d[tn * P:(tn + 1) * P,
                                    (en - 1) * IC:(en + 1) * IC],
                                stage[:, (en - 1) * IC:(en + 1) * IC])

                # ---- schedule ----
                # batch 0, head 0: no out-projection available yet
                for icn in range(NIC):
                    attn_ic(0, 0, icn * IC, IC, vbh00)
                # batch 0, head 1: tn groups 0..5 become available as the
                # at-slices complete (tn k needs both heads for tokens
                # [k*128,(k+1)*128), i.e. b0h1 ic >= k//4 done)
                vbh = load_vbh(0, 1)
                for icn in range(NIC):
                    attn_ic(0, 1, icn * IC, IC, vbh)
                    if icn >= 1:
                        for k in range(2):
                            ph3_tn((icn - 1) * 2 + k)
                # batch 1, head 0: finish batch-0 out-projection (tn 6..15)
                vbh = load_vbh(1, 0)
                b0_rest = list(range(6, 16))
                for icn in range(NIC):
                    attn_ic(1, 0, icn * IC, IC, vbh)
                    take = 3 if icn < 2 else 2
                    for _ in range(take):
                        if b0_rest:
                            ph3_tn(b0_rest.pop(0))
                for tn in b0_rest:
                    ph3_tn(tn)
                # batch 1, head 1: batch-1 tn groups become available per ic;
                # the last 512-query chunk is split in half so the final
                # out-projection chunks start (and the tail ends) earlier
                vbh = load_vbh(1, 1)
                for icn in range(NIC - 1):
                    attn_ic(1, 1, icn * IC, IC, vbh)
                    if icn > 0:
                        for k in range(4):
                            ph3_tn(16 + (icn - 1) * 4 + k)
                attn_ic(1, 1, 3 * IC, IC // 2, vbh)
                for k in range(4):
                    ph3_tn(16 + 2 * 4 + k)
                attn_ic(1, 1, 3 * IC + IC // 2, IC // 2, vbh)
                for tn in (28, 29, 30, 31):
                    ph3_tn(tn)

    nc.finalize()
    return nc


def _rope_tables():
    """cos/sin tables in [head_dim, token] layout, matching the reference's
    f32 computation (jax on CPU when available)."""
    try:
        import jax
        import jax.numpy as jnp
        cpu = jax.devices("cpu")[0]
        with jax.default_device(cpu):
            inv = 1.0 / (10000.0 ** (
                jnp.arange(0, P, 2, dtype=jnp.float32) / P))
            t = jnp.arange(S, dtype=jnp.float32)
            freqs = jnp.einsum("i,j->ij", t, inv)          # [S, 64]
            emb = jnp.concatenate((freqs, freqs), axis=-1)  # [S, 128]
            cos = np.asarray(jnp.cos(emb)).T                # [128, S]
            sin = np.asarray(jnp.sin(emb)).T
    except Exception:
        inv = 1.0 / (10000.0 ** (np.arange(0, P, 2, dtype=np.float64) / P))
        t = np.arange(S, dtype=np.float64)
        freqs = np.outer(t, inv)
        emb = np.concatenate((freqs, freqs), axis=-1)
        cos = np.cos(emb).T.astype(np.float32)
        sin = np.sin(emb).T.astype(np.float32)

    cos2 = np.ascontiguousarray(np.tile(cos, (1, B)).astype(np.float32))
    sin_z = np.concatenate([sin[0:64], -sin[64:128]], axis=0)
    sin2 = np.ascontiguousarray(np.tile(sin_z, (1, B)).astype(np.float32))
    return cos2, sin2


_NC_CACHE = None


def _rearr_w(w):
    """[DIM, m] -> [P, KC, m] with partition-contiguous rows."""
    m = w.shape[1]
    return np.ascontiguousarray(
        w.reshape(KC, P, m).transpose(1, 0, 2)).astype(MM_NP)


def _in_maps(x, Wq, Wk, Wv, Wo):
    # [NTC, P, KC, TC]: xT[n, p, c, t] = x^T[c*P+p, n*TC+t] -- each token
    # chunk is one fully-contiguous 16KB-per-partition DMA
    xT = np.ascontiguousarray(
        x.reshape(TOK, DIM).T.reshape(KC, P, NTC, TC).transpose(2, 1, 0, 3)
    ).astype(MM_NP)
    cosT, sinZ = _rope_tables()
    ones = np.ones((P, P), dtype=AVD_NP)
    maps = []
    for c in range(N_CORES):
        cs = slice(c * INNER_C, (c + 1) * INNER_C)
        maps.append({
            "xT": xT,
            "wq": _rearr_w(Wq[:, cs]),
            "wk": _rearr_w(Wk[:, cs]),
            "wv": _rearr_w(Wv[:, cs]),
            "wo": np.ascontiguousarray(
                Wo[cs, :].reshape(2, P, DIM).transpose(1, 0, 2)
            ).astype(np.float16),
            "cosT": cosT,
            "sinZ": sinZ,
            "ones": ones,
        })
    return maps


def kernel(x, Wq, Wk, Wv, Wo):
    global _NC_CACHE
    assert x.shape == (B, S, DIM)
    if _NC_CACHE is None:
        _NC_CACHE = _build()
    in_maps = _in_maps(x, Wq, Wk, Wv, Wo)
    last_err = None
    for attempt in range(3):
        try:
            res = run_bass_kernel_spmd(_NC_CACHE, in_maps,
                                       core_ids=list(range(N_CORES)),
                                       trace=False)
            break
        except Exception as e:  # transient NRT faults: retry
            last_err = e
    else:
        raise last_err
    out = res.results[0]["o_part"].astype(np.float32)
    for c in range(1, N_CORES):
        out += res.results[c]["o_part"].astype(np.float32)
    return out.astype(np.float32).reshape(B, S, DIM)
